# revision 5
# baseline (speedup 1.0000x reference)
"""AdaAttention Trainium2 kernel — data-parallel over batch across 8 NeuronCores.

Full shapes: h [1024,512], sentinel [1024,512], att_feats [1024,96,2048] -> out [1024,512].
Per core: b=128 batch rows. Token axis x = s*128 + b (s-major), N_tok = 12288.

Per-core pipeline (matmuls bf16, psum f32):
  att_feats --SWDGE cast f32->bf16--> nat[b,2048] --xbar transpose--> attf_T[f,x]
  MM1: attT[r,x] = W_aeT.T @ attf_T ; ACT relu(+b_ae) -> bf16
  MM2: att_embdT[a,x] = W_cT.T @ attT ; DVE +h_eT bcast ; ACT tanh(+b_c) -> hAT bf16
  logits row = w_al.T @ hAT -> PE col-transposes -> logits[b,s]
  xbar: attT -> att[x,r] tiles (for cHat)
  sentinel/h prep: PE transposes, sent_eT/h_eT MMs, hA_sent, sentinel logit
  softmax(f32) -> alpha[b,97]; Msel_s = diag(alpha[:,s]) via DVE
  cHat = sum_s Msel_s.T @ att_s (+ sentinel term)  [97 accumulating MMs]
  out = tanh((cHat + h) @ W_oT + b_o) via PE transposes + MM; f32 out.
b_al is skipped everywhere: softmax is invariant to a constant logit shift.
"""
import sys

for p in ("/opt/trn_rl_repo", "/opt/pypackages"):
    if p not in sys.path:
        sys.path.insert(0, p)

import numpy as np
import ml_dtypes
from contextlib import ExitStack

import concourse.bass as bass
import concourse.bacc as bacc
import concourse.mybir as mybir
from concourse import tile

F32 = mybir.dt.float32
BF16 = mybir.dt.bfloat16
AF = mybir.ActivationFunctionType
ALU = mybir.AluOpType

NCORES = 8
B_LOC = 128          # batch rows per core
S = 96               # attention slots
F = 2048             # att feature size
R = 512              # rnn size
A = 512              # att hidden size
NTOK = B_LOC * S     # 12288
XCHUNK = 512         # tokens per pipeline chunk (4 s-tiles)
NCHUNKS = NTOK // XCHUNK       # 24
S_PER_CHUNK = XCHUNK // B_LOC  # 4
FT = F // 128        # 16 f-tiles
RT = R // 128        # 4
AT = A // 128        # 4


def build_nc():
    nc = bacc.Bacc("TRN2", target_bir_lowering=False, debug=False)

    # ---- DRAM parameters (per-core shard shapes) ----
    att_feats = nc.declare_dram_parameter("att_feats", [B_LOC, S, F], F32, isOutput=False)
    h_in = nc.declare_dram_parameter("h", [B_LOC, R], F32, isOutput=False)
    sent_in = nc.declare_dram_parameter("sentinel", [B_LOC, R], F32, isOutput=False)
    # host-prepped weights (bf16, pre-transposed): [p, t, n] = W.T[128*t + p, n]
    w_ae_t = nc.declare_dram_parameter("w_ae_t", [128, FT, R], BF16, isOutput=False)
    w_c_t = nc.declare_dram_parameter("w_c_t", [128, RT, A], BF16, isOutput=False)
    w_s_t = nc.declare_dram_parameter("w_s_t", [128, RT, A], BF16, isOutput=False)
    w_h_t = nc.declare_dram_parameter("w_h_t", [128, RT, A], BF16, isOutput=False)
    w_o_t = nc.declare_dram_parameter("w_o_t", [128, RT, R], BF16, isOutput=False)
    w_al_d = nc.declare_dram_parameter("w_al", [128, AT], BF16, isOutput=False)
    b_ae_d = nc.declare_dram_parameter("b_ae", [128, RT], F32, isOutput=False)
    b_c_d = nc.declare_dram_parameter("b_c", [128, AT], F32, isOutput=False)
    b_s_d = nc.declare_dram_parameter("b_s", [128, AT], F32, isOutput=False)
    b_h_d = nc.declare_dram_parameter("b_h", [128, AT], F32, isOutput=False)
    b_o_d = nc.declare_dram_parameter("b_o", [128, RT], F32, isOutput=False)
    ident_d = nc.declare_dram_parameter("ident", [128, 128], BF16, isOutput=False)
    ident_f32_d = nc.declare_dram_parameter("ident_f32", [128, 128], F32, isOutput=False)
    out_d = nc.declare_dram_parameter("out", [B_LOC, R], F32, isOutput=True)

    with tile.TileContext(nc) as tc, ExitStack() as ctx:
        # ---- pools ----
        cp = ctx.enter_context(tc.tile_pool(name="consts", bufs=1))
        nat_p = ctx.enter_context(tc.tile_pool(name="nat", bufs=3))
        attf_p = ctx.enter_context(tc.tile_pool(name="attf", bufs=2))
        attT_p = ctx.enter_context(tc.tile_pool(name="attT", bufs=6))
        hat_p = ctx.enter_context(tc.tile_pool(name="hat", bufs=6))
        attx_p = ctx.enter_context(tc.tile_pool(name="attx", bufs=S))
        small_p = ctx.enter_context(tc.tile_pool(name="small", bufs=2))
        soft_p = ctx.enter_context(tc.tile_pool(name="soft", bufs=3))
        msel_p = ctx.enter_context(tc.tile_pool(name="msel", bufs=4))
        ps_mm1 = ctx.enter_context(tc.tile_pool(name="ps_mm1", bufs=3, space="PSUM"))
        ps_mm2 = ctx.enter_context(tc.tile_pool(name="ps_mm2", bufs=2, space="PSUM"))
        ps_small = ctx.enter_context(tc.tile_pool(name="ps_small", bufs=2, space="PSUM"))
        ps_chat = ctx.enter_context(tc.tile_pool(name="ps_chat", bufs=1, space="PSUM"))

        def const_tile(name, shape, dtype, src):
            t = cp.tile(shape, dtype, tag=name, name=name)
            nc.sync.dma_start(out=t[:], in_=src[:])
            return t

        # ---- constants / weights ----
        ident = const_tile("ident", [128, 128], BF16, ident_d)
        ident_f32 = const_tile("ident_f32", [128, 128], F32, ident_f32_d)
        w_ae = const_tile("w_ae", [128, FT, R], BF16, w_ae_t)
        w_c = const_tile("w_c", [128, RT, A], BF16, w_c_t)
        w_s = const_tile("w_s", [128, RT, A], BF16, w_s_t)
        w_h = const_tile("w_h", [128, RT, A], BF16, w_h_t)
        w_o = const_tile("w_o", [128, RT, R], BF16, w_o_t)
        wal = const_tile("wal", [128, AT], BF16, w_al_d)
        b_ae = const_tile("b_ae", [128, RT], F32, b_ae_d)
        b_c = const_tile("b_c", [128, AT], F32, b_c_d)
        b_s = const_tile("b_s", [128, AT], F32, b_s_d)
        b_h = const_tile("b_h", [128, AT], F32, b_h_d)
        b_o = const_tile("b_o", [128, RT], F32, b_o_d)

        # ---- h / sentinel prep ----
        h_f32 = const_tile("h_f32", [B_LOC, R], F32, h_in)
        h_bf = cp.tile([B_LOC, R], BF16, tag="h_bf", name="h_bf")
        nc.vector.tensor_copy(h_bf[:], h_f32[:])
        sent_bf = cp.tile([B_LOC, R], BF16, tag="sent_bf", name="sent_bf")
        nc.gpsimd.dma_start(out=sent_bf[:], in_=sent_in[:])  # cast f32->bf16 in DMA

        hT = cp.tile([128, RT, B_LOC], BF16, tag="hT", name="hT")
        sentT = cp.tile([128, RT, B_LOC], BF16, tag="sentT", name="sentT")
        for rb in range(RT):
            pt = ps_small.tile([128, 128], BF16, tag="pssm", name="pt_h")
            nc.tensor.transpose(pt[:], h_bf[:, rb * 128:(rb + 1) * 128], ident[:])
            nc.vector.tensor_copy(hT[:, rb, :], pt[:])
            pt2 = ps_small.tile([128, 128], BF16, tag="pssm", name="pt_s")
            nc.tensor.transpose(pt2[:], sent_bf[:, rb * 128:(rb + 1) * 128], ident[:])
            nc.vector.tensor_copy(sentT[:, rb, :], pt2[:])

        # h_eT[a, b] = (h @ W_hT).T + b_h   [128p(a), AT, 128b] bf16
        h_eT = cp.tile([128, AT, B_LOC], BF16, tag="h_eT", name="h_eT")
        for ab in range(AT):
            psh = ps_small.tile([128, B_LOC], F32, tag="pssm", name="psh")
            for rb in range(RT):
                nc.tensor.matmul(psh[:], w_h[:, rb, ab * 128:(ab + 1) * 128], hT[:, rb, :],
                                 start=(rb == 0), stop=(rb == RT - 1))
            nc.scalar.activation(h_eT[:, ab, :], psh[:], AF.Identity,
                                 bias=b_h[:, ab:ab + 1], scale=1.0)

        # hA_sentT = tanh(sent_eT + h_eT + b_s)   [128p(a), AT, 128b] bf16
        hA_sentT = cp.tile([128, AT, B_LOC], BF16, tag="hA_sentT", name="hA_sentT")
        for ab in range(AT):
            pss = ps_small.tile([128, B_LOC], F32, tag="pssm", name="pss")
            for rb in range(RT):
                nc.tensor.matmul(pss[:], w_s[:, rb, ab * 128:(ab + 1) * 128], sentT[:, rb, :],
                                 start=(rb == 0), stop=(rb == RT - 1))
            tmp = small_p.tile([128, B_LOC], F32, tag="preptmp", name="prep_tmp")
            nc.vector.tensor_tensor(out=tmp[:], in0=pss[:], in1=h_eT[:, ab, :], op=ALU.add)
            nc.scalar.activation(hA_sentT[:, ab, :], tmp[:], AF.Tanh,
                                 bias=b_s[:, ab:ab + 1], scale=1.0)

        # sentinel logit -> logits_sb[:, 0]
        logits_sb = cp.tile([B_LOC, 1 + S], F32, tag="logits", name="logits_sb")
        ps_lr0 = ps_small.tile([1, B_LOC], F32, tag="pssm", name="ps_lr0")
        for ab in range(AT):
            nc.tensor.matmul(ps_lr0[:], wal[:, ab:ab + 1], hA_sentT[:, ab, :],
                             start=(ab == 0), stop=(ab == AT - 1))
        lrow0 = small_p.tile([1, B_LOC], F32, tag="lrow", name="lrow0")
        nc.vector.tensor_copy(lrow0[:], ps_lr0[:])
        ps_lc0 = ps_small.tile([128, 1], F32, tag="pssm", name="ps_lc0")
        nc.tensor.transpose(ps_lc0[:], lrow0[:], ident_f32[0:1, 0:1])
        nc.vector.tensor_copy(logits_sb[:, 0:1], ps_lc0[:])

        # ---- main x-chunk pipeline ----
        att_x_tiles = []
        for c in range(NCHUNKS):
            # 1. input DMA (SWDGE, f32->bf16 cast), one per s-tile
            nats = []
            for i in range(S_PER_CHUNK):
                s_idx = c * S_PER_CHUNK + i
                nat = nat_p.tile([B_LOC, F], BF16, tag="nat", name=f"nat_{s_idx}")
                nc.gpsimd.dma_start(out=nat[:], in_=att_feats[:, s_idx, :])
                nats.append(nat)

            # 2. xbar transpose -> attf [128p, FT, XCHUNK]; slice c holds f rows 128c..128c+127
            attf = attf_p.tile([128, FT, XCHUNK], BF16, tag="attf", name=f"attf_{c}")
            for i in range(S_PER_CHUNK):
                nc.sync.dma_start(out=attf[:, :, i * 128:(i + 1) * 128],
                                  in_=nats[i][:], transpose=True)

            # 3. MM1 + relu -> attT tiles [128p(r), XCHUNK] bf16
            attT_tiles = []
            for rb in range(RT):
                ps1 = ps_mm1.tile([128, XCHUNK], F32, tag="mm1", name=f"ps1_{c}_{rb}")
                for f in range(FT):
                    nc.tensor.matmul(ps1[:], w_ae[:, f, rb * 128:(rb + 1) * 128],
                                     attf[:, f, :], start=(f == 0), stop=(f == FT - 1))
                at_sb = attT_p.tile([128, XCHUNK], BF16, tag="attT", name=f"attT_{c}_{rb}")
                nc.scalar.activation(at_sb[:], ps1[:], AF.Relu,
                                     bias=b_ae[:, rb:rb + 1], scale=1.0)
                attT_tiles.append(at_sb)

            # 4. MM2 -> +h_eT bcast -> tanh(+b_c) -> hAT bf16
            hat_tiles = []
            for ab in range(AT):
                ps2 = ps_mm2.tile([128, XCHUNK], F32, tag="mm2", name=f"ps2_{c}_{ab}")
                for rb in range(RT):
                    nc.tensor.matmul(ps2[:], w_c[:, rb, ab * 128:(ab + 1) * 128],
                                     attT_tiles[rb][:], start=(rb == 0), stop=(rb == RT - 1))
                tmp = small_p.tile([128, XCHUNK], BF16, tag="hatmp", name=f"hatmp_{c}_{ab}")
                nc.vector.tensor_tensor(
                    out=tmp[:].rearrange("p (s b) -> p s b", s=S_PER_CHUNK),
                    in0=ps2[:].rearrange("p (s b) -> p s b", s=S_PER_CHUNK),
                    in1=h_eT[:, ab, :].unsqueeze(1).broadcast_to([128, S_PER_CHUNK, B_LOC]),
                    op=ALU.add)
                ht = hat_p.tile([128, XCHUNK], BF16, tag="hat", name=f"hat_{c}_{ab}")
                nc.scalar.activation(ht[:], tmp[:], AF.Tanh,
                                     bias=b_c[:, ab:ab + 1], scale=1.0)
                hat_tiles.append(ht)

            # 5. logits row -> col transposes -> logits_sb[:, 1+4c : 1+4c+4]
            ps_l = ps_small.tile([1, XCHUNK], F32, tag="pssm", name=f"ps_l_{c}")
            for ab in range(AT):
                nc.tensor.matmul(ps_l[:], wal[:, ab:ab + 1], hat_tiles[ab][:],
                                 start=(ab == 0), stop=(ab == AT - 1))
            lr = small_p.tile([1, XCHUNK], F32, tag="lrow", name=f"lr_{c}")
            nc.vector.tensor_copy(lr[:], ps_l[:])
            ps_cc = ps_small.tile([128, S_PER_CHUNK], F32, tag="pssm", name=f"ps_cc_{c}")
            for i in range(S_PER_CHUNK):
                nc.tensor.transpose(ps_cc[:, i:i + 1], lr[:, i * 128:(i + 1) * 128],
                                    ident_f32[0:1, 0:1])
            nc.vector.tensor_copy(
                logits_sb[:, 1 + c * S_PER_CHUNK: 1 + (c + 1) * S_PER_CHUNK], ps_cc[:])

            # 6. xbar attT -> att[x, r] persistent tiles (for cHat)
            for i in range(S_PER_CHUNK):
                ax = attx_p.tile([128, R], BF16, tag="attx", name=f"attx_{c}_{i}")
                for rb in range(RT):
                    nc.sync.dma_start(out=ax[:, rb * 128:(rb + 1) * 128],
                                      in_=attT_tiles[rb][:, i * 128:(i + 1) * 128],
                                      transpose=True)
                att_x_tiles.append(ax)

        # ---- softmax over 97 slots (f32) ----
        mx = soft_p.tile([B_LOC, 1], F32, tag="soft", name="mx")
        nc.vector.tensor_reduce(out=mx[:], in_=logits_sb[:], op=ALU.max,
                                axis=mybir.AxisListType.X)
        shifted = soft_p.tile([B_LOC, 1 + S], F32, tag="soft", name="shifted")
        nc.vector.tensor_scalar(out=shifted[:], in0=logits_sb[:], scalar1=mx[:],
                                scalar2=None, op0=ALU.subtract)
        expd = soft_p.tile([B_LOC, 1 + S], F32, tag="soft", name="expd")
        nc.scalar.activation(expd[:], shifted[:], AF.Exp)
        ssum = soft_p.tile([B_LOC, 1], F32, tag="soft", name="ssum")
        nc.vector.tensor_reduce(out=ssum[:], in_=expd[:], op=ALU.add,
                                axis=mybir.AxisListType.X)
        rin = soft_p.tile([B_LOC, 1], F32, tag="soft", name="rin")
        nc.vector.reciprocal(rin[:], ssum[:])
        alpha = cp.tile([B_LOC, 1 + S], F32, tag="alpha", name="alpha")
        nc.vector.tensor_scalar(out=alpha[:], in0=expd[:], scalar1=rin[:],
                                scalar2=None, op0=ALU.mult)

        # ---- cHat: 97 accumulating diag matmuls ----
        ps_cH = ps_chat.tile([B_LOC, R], F32, name="ps_cH")
        ms0 = msel_p.tile([128, 128], BF16, tag="msel", name="ms0")
        nc.vector.tensor_scalar(out=ms0[:], in0=ident[:], scalar1=alpha[:, 0:1],
                                scalar2=None, op0=ALU.mult)
        nc.tensor.matmul(ps_cH[:], ms0[:], sent_bf[:], start=True, stop=False)
        for t in range(S):
            ms = msel_p.tile([128, 128], BF16, tag="msel", name=f"ms_{t}")
            nc.vector.tensor_scalar(out=ms[:], in0=ident[:], scalar1=alpha[:, t + 1:t + 2],
                                    scalar2=None, op0=ALU.mult)
            nc.tensor.matmul(ps_cH[:], ms[:], att_x_tiles[t][:],
                             start=False, stop=(t == S - 1))

        # ---- final: out = tanh((cHat + h) @ W_oT + b_o) ----
        atten_bf = cp.tile([B_LOC, R], BF16, tag="atten", name="atten_bf")
        nc.vector.tensor_tensor(out=atten_bf[:], in0=ps_cH[:], in1=h_f32[:], op=ALU.add)
        attenT = cp.tile([128, RT, B_LOC], BF16, tag="attenT", name="attenT")
        for rb in range(RT):
            ptf = ps_small.tile([128, 128], BF16, tag="pssm", name=f"ptf_{rb}")
            nc.tensor.transpose(ptf[:], atten_bf[:, rb * 128:(rb + 1) * 128], ident[:])
            nc.vector.tensor_copy(attenT[:, rb, :], ptf[:])
        for ob in range(RT):
            pso = ps_small.tile([128, B_LOC], F32, tag="pssm", name=f"pso_{ob}")
            for rb in range(RT):
                nc.tensor.matmul(pso[:], w_o[:, rb, ob * 128:(ob + 1) * 128], attenT[:, rb, :],
                                 start=(rb == 0), stop=(rb == RT - 1))
            otmp = small_p.tile([128, B_LOC], F32, tag="otmp", name=f"otmp_{ob}")
            nc.scalar.activation(otmp[:], pso[:], AF.Tanh,
                                 bias=b_o[:, ob:ob + 1], scale=1.0)
            ptb = ps_small.tile([128, 128], F32, tag="pssm", name=f"ptb_{ob}")
            nc.tensor.transpose(ptb[:], otmp[:], ident_f32[:])
            ostg = small_p.tile([128, 128], F32, tag="ostg", name=f"ostg_{ob}")
            nc.vector.tensor_copy(ostg[:], ptb[:])
            nc.sync.dma_start(out=out_d[:, ob * 128:(ob + 1) * 128], in_=ostg[:])

    nc.compile()
    return nc


# ---------------- host side ----------------
_NC_CACHE = None


def _get_nc():
    global _NC_CACHE
    if _NC_CACHE is None:
        _NC_CACHE = build_nc()
    return _NC_CACHE


def prep_shared(W_ae, b_ae, W_c, b_c, W_s, b_s, W_h, b_h, W_al, b_al, W_o, b_o):
    bf = ml_dtypes.bfloat16

    def wt(w, nt):  # [p, t, n] = w.T[128*t + p, n]
        wT = np.ascontiguousarray(np.asarray(w, np.float32).T)
        return np.ascontiguousarray(
            wT.reshape(nt, 128, wT.shape[1]).transpose(1, 0, 2)).astype(bf)

    def bt(b, nt):  # [p, t] = b[128*t + p]
        return np.ascontiguousarray(
            np.asarray(b, np.float32).reshape(nt, 128).T).astype(np.float32)

    return {
        "w_ae_t": wt(W_ae, FT),
        "w_c_t": wt(W_c, RT),
        "w_s_t": wt(W_s, RT),
        "w_h_t": wt(W_h, RT),
        "w_o_t": wt(W_o, RT),
        "w_al": np.ascontiguousarray(
            np.asarray(W_al, np.float32)[0].reshape(AT, 128).T).astype(bf),
        "b_ae": bt(b_ae, RT),
        "b_c": bt(b_c, AT),
        "b_s": bt(b_s, AT),
        "b_h": bt(b_h, AT),
        "b_o": bt(b_o, RT),
        "ident": np.eye(128, dtype=bf),
        "ident_f32": np.eye(128, dtype=np.float32),
    }


def make_in_maps(h, sentinel, att_feats, shared):
    h = np.asarray(h, np.float32)
    sentinel = np.asarray(sentinel, np.float32)
    att_feats = np.asarray(att_feats, np.float32)
    in_maps = []
    for i in range(NCORES):
        sl = slice(i * B_LOC, (i + 1) * B_LOC)
        m = dict(shared)
        m["h"] = np.ascontiguousarray(h[sl])
        m["sentinel"] = np.ascontiguousarray(sentinel[sl])
        m["att_feats"] = np.ascontiguousarray(att_feats[sl])
        in_maps.append(m)
    return in_maps


def kernel(h, sentinel, att_feats, W_ae, b_ae, W_c, b_c, W_s, b_s,
           W_h, b_h, W_al, b_al, W_o, b_o):
    shared = prep_shared(W_ae, b_ae, W_c, b_c, W_s, b_s, W_h, b_h, W_al, b_al, W_o, b_o)
    in_maps = make_in_maps(h, sentinel, att_feats, shared)
    nc = _get_nc()
    from concourse.bass_utils import run_bass_kernel_spmd
    res = run_bass_kernel_spmd(nc, in_maps, core_ids=list(range(NCORES)))
    out = np.concatenate([res.results[i]["out"] for i in range(NCORES)], axis=0)
    return np.ascontiguousarray(out.astype(np.float32))


if __name__ == "__main__":
    build_nc()
    print("built ok")


# revision 10
# speedup vs baseline: 1.2756x; 1.2756x over previous
"""AdaAttention Trainium2 kernel — data-parallel over batch across 8 NeuronCores.

Full shapes: h [1024,512], sentinel [1024,512], att_feats [1024,96,2048] -> out [1024,512].
Per core: b=128 batch rows. Token axis x = s*128 + b (s-major), N_tok = 12288.

Per-core pipeline (matmuls bf16, psum f32):
  att_feats --SWDGE cast f32->bf16--> nat[b,2048] --xbar transpose--> attf_T[f,x]
  MM1: attT[r,x] = W_aeT.T @ attf_T ; ACT relu(+b_ae) -> bf16
  MM2: att_embdT[a,x] = W_cT.T @ attT ; DVE +h_eT bcast ; ACT tanh(+b_c) -> hAT bf16
  logits row = w_al.T @ hAT -> PE col-transposes -> logits[b,s]
  xbar: attT -> att[x,r] tiles (for cHat)
  sentinel/h prep: PE transposes, sent_eT/h_eT MMs, hA_sent, sentinel logit
  softmax(f32) -> alpha[b,97]; Msel_s = diag(alpha[:,s]) via DVE
  cHat = sum_s Msel_s.T @ att_s (+ sentinel term)  [97 accumulating MMs]
  out = tanh((cHat + h) @ W_oT + b_o) via PE transposes + MM; f32 out.
b_al is skipped everywhere: softmax is invariant to a constant logit shift.
"""
import sys

for p in ("/opt/trn_rl_repo", "/opt/pypackages"):
    if p not in sys.path:
        sys.path.insert(0, p)

import numpy as np
import ml_dtypes
from contextlib import ExitStack

import concourse.bass as bass
import concourse.bacc as bacc
import concourse.mybir as mybir
from concourse import tile

F32 = mybir.dt.float32
BF16 = mybir.dt.bfloat16
AF = mybir.ActivationFunctionType
ALU = mybir.AluOpType

NCORES = 8
B_LOC = 128          # batch rows per core
S = 96               # attention slots
F = 2048             # att feature size
R = 512              # rnn size
A = 512              # att hidden size
NTOK = B_LOC * S     # 12288
XCHUNK = 512         # tokens per pipeline chunk (4 s-tiles)
NCHUNKS = NTOK // XCHUNK       # 24
S_PER_CHUNK = XCHUNK // B_LOC  # 4
FT = F // 128        # 16 f-tiles
RT = R // 128        # 4
AT = A // 128        # 4


def build_nc():
    nc = bacc.Bacc("TRN2", target_bir_lowering=False, debug=False)

    # ---- DRAM parameters (per-core shard shapes) ----
    att_feats = nc.declare_dram_parameter("att_feats", [B_LOC, S, F], F32, isOutput=False)
    h_in = nc.declare_dram_parameter("h", [B_LOC, R], F32, isOutput=False)
    sent_in = nc.declare_dram_parameter("sentinel", [B_LOC, R], F32, isOutput=False)
    # host-prepped weights (bf16, pre-transposed): [p, t, n] = W.T[128*t + p, n]
    w_ae_t = nc.declare_dram_parameter("w_ae_t", [128, FT, R], BF16, isOutput=False)
    w_c_t = nc.declare_dram_parameter("w_c_t", [128, RT, A], BF16, isOutput=False)
    w_s_t = nc.declare_dram_parameter("w_s_t", [128, RT, A], BF16, isOutput=False)
    w_h_t = nc.declare_dram_parameter("w_h_t", [128, RT, A], BF16, isOutput=False)
    w_o_t = nc.declare_dram_parameter("w_o_t", [128, RT, R], BF16, isOutput=False)
    w_al_d = nc.declare_dram_parameter("w_al", [128, AT], BF16, isOutput=False)
    b_ae_d = nc.declare_dram_parameter("b_ae", [128, RT], F32, isOutput=False)
    b_c_d = nc.declare_dram_parameter("b_c", [128, AT], F32, isOutput=False)
    b_s_d = nc.declare_dram_parameter("b_s", [128, AT], F32, isOutput=False)
    b_h_d = nc.declare_dram_parameter("b_h", [128, AT], F32, isOutput=False)
    b_o_d = nc.declare_dram_parameter("b_o", [128, RT], F32, isOutput=False)
    ident_d = nc.declare_dram_parameter("ident", [128, 128], BF16, isOutput=False)
    ident_f32_d = nc.declare_dram_parameter("ident_f32", [128, 128], F32, isOutput=False)
    out_d = nc.declare_dram_parameter("out", [B_LOC, R], F32, isOutput=True)

    with tile.TileContext(nc) as tc, ExitStack() as ctx:
        # ---- pools ----
        cp = ctx.enter_context(tc.tile_pool(name="consts", bufs=1))
        nat_p = ctx.enter_context(tc.tile_pool(name="nat", bufs=3))
        attf_p = ctx.enter_context(tc.tile_pool(name="attf", bufs=2))
        attT_p = ctx.enter_context(tc.tile_pool(name="attT", bufs=5))
        hat_p = ctx.enter_context(tc.tile_pool(name="hat", bufs=5))
        attx_p = ctx.enter_context(tc.tile_pool(name="attx", bufs=NCHUNKS))
        small_p = ctx.enter_context(tc.tile_pool(name="small", bufs=2))
        soft_p = ctx.enter_context(tc.tile_pool(name="soft", bufs=3))
        msel_p = ctx.enter_context(tc.tile_pool(name="msel", bufs=4))
        ps_mm1 = ctx.enter_context(tc.tile_pool(name="ps_mm1", bufs=3, space="PSUM"))
        ps_mm2 = ctx.enter_context(tc.tile_pool(name="ps_mm2", bufs=2, space="PSUM"))
        ps_small = ctx.enter_context(tc.tile_pool(name="ps_small", bufs=2, space="PSUM"))
        ps_chat = ctx.enter_context(tc.tile_pool(name="ps_chat", bufs=1, space="PSUM"))

        def const_tile(name, shape, dtype, src):
            t = cp.tile(shape, dtype, tag=name, name=name)
            nc.gpsimd.dma_start(out=t[:], in_=src[:])
            return t

        # ---- constants / weights ----
        ident = const_tile("ident", [128, 128], BF16, ident_d)
        ident_f32 = const_tile("ident_f32", [128, 128], F32, ident_f32_d)
        w_ae = const_tile("w_ae", [128, FT, R], BF16, w_ae_t)
        w_c = const_tile("w_c", [128, RT, A], BF16, w_c_t)
        w_s = const_tile("w_s", [128, RT, A], BF16, w_s_t)
        w_h = const_tile("w_h", [128, RT, A], BF16, w_h_t)
        w_o = const_tile("w_o", [128, RT, R], BF16, w_o_t)
        wal = const_tile("wal", [128, AT], BF16, w_al_d)
        b_ae = const_tile("b_ae", [128, RT], F32, b_ae_d)
        b_c = const_tile("b_c", [128, AT], F32, b_c_d)
        b_s = const_tile("b_s", [128, AT], F32, b_s_d)
        b_h = const_tile("b_h", [128, AT], F32, b_h_d)
        b_o = const_tile("b_o", [128, RT], F32, b_o_d)

        # ---- h / sentinel prep ----
        h_f32 = const_tile("h_f32", [B_LOC, R], F32, h_in)
        h_bf = cp.tile([B_LOC, R], BF16, tag="h_bf", name="h_bf")
        nc.vector.tensor_copy(h_bf[:], h_f32[:])
        sent_bf = cp.tile([B_LOC, R], BF16, tag="sent_bf", name="sent_bf")
        nc.gpsimd.dma_start(out=sent_bf[:], in_=sent_in[:])  # cast f32->bf16 in DMA

        hT = cp.tile([128, RT, B_LOC], BF16, tag="hT", name="hT")
        sentT = cp.tile([128, RT, B_LOC], BF16, tag="sentT", name="sentT")
        for rb in range(RT):
            pt = ps_small.tile([128, 128], BF16, tag="pssm", name="pt_h")
            nc.tensor.transpose(pt[:], h_bf[:, rb * 128:(rb + 1) * 128], ident[:])
            nc.vector.tensor_copy(hT[:, rb, :], pt[:])
            pt2 = ps_small.tile([128, 128], BF16, tag="pssm", name="pt_s")
            nc.tensor.transpose(pt2[:], sent_bf[:, rb * 128:(rb + 1) * 128], ident[:])
            nc.vector.tensor_copy(sentT[:, rb, :], pt2[:])

        # h_eT[a, b] = (h @ W_hT).T + b_h   [128p(a), AT, 128b] bf16
        h_eT = cp.tile([128, AT, B_LOC], BF16, tag="h_eT", name="h_eT")
        for ab in range(AT):
            psh = ps_small.tile([128, B_LOC], F32, tag="pssm", name="psh")
            for rb in range(RT):
                nc.tensor.matmul(psh[:], w_h[:, rb, ab * 128:(ab + 1) * 128], hT[:, rb, :],
                                 start=(rb == 0), stop=(rb == RT - 1))
            nc.scalar.activation(h_eT[:, ab, :], psh[:], AF.Identity,
                                 bias=b_h[:, ab:ab + 1], scale=1.0)

        # hA_sentT = tanh(sent_eT + h_eT + b_s)   [128p(a), AT, 128b] bf16
        hA_sentT = cp.tile([128, AT, B_LOC], BF16, tag="hA_sentT", name="hA_sentT")
        for ab in range(AT):
            pss = ps_small.tile([128, B_LOC], F32, tag="pssm", name="pss")
            for rb in range(RT):
                nc.tensor.matmul(pss[:], w_s[:, rb, ab * 128:(ab + 1) * 128], sentT[:, rb, :],
                                 start=(rb == 0), stop=(rb == RT - 1))
            tmp = small_p.tile([128, B_LOC], F32, tag="preptmp", name="prep_tmp")
            nc.vector.tensor_tensor(out=tmp[:], in0=pss[:], in1=h_eT[:, ab, :], op=ALU.add)
            nc.scalar.activation(hA_sentT[:, ab, :], tmp[:], AF.Tanh,
                                 bias=b_s[:, ab:ab + 1], scale=1.0)

        # sentinel logit -> logits_sb[:, 0]
        logits_sb = cp.tile([B_LOC, 1 + S], F32, tag="logits", name="logits_sb")
        ps_lr0 = ps_small.tile([1, B_LOC], F32, tag="pssm", name="ps_lr0")
        for ab in range(AT):
            nc.tensor.matmul(ps_lr0[:], wal[:, ab:ab + 1], hA_sentT[:, ab, :],
                             start=(ab == 0), stop=(ab == AT - 1))
        lrow0 = small_p.tile([1, B_LOC], F32, tag="lrow", name="lrow0")
        nc.vector.tensor_copy(lrow0[:], ps_lr0[:])
        ps_lc0 = ps_small.tile([128, 1], F32, tag="pssm", name="ps_lc0")
        nc.tensor.transpose(ps_lc0[:], lrow0[:], ident_f32[0:1, 0:1])
        nc.vector.tensor_copy(logits_sb[:, 0:1], ps_lc0[:])

        # ---- main x-chunk pipeline ----
        att_x_chunks = []
        for c in range(NCHUNKS):
            # 1. input DMA (SWDGE, f32->bf16 cast), one per s-tile
            nats = []
            for i in range(S_PER_CHUNK):
                s_idx = c * S_PER_CHUNK + i
                nat = nat_p.tile([B_LOC, F], BF16, tag="nat", name=f"nat_{s_idx}")
                nc.gpsimd.dma_start(out=nat[:], in_=att_feats[:, s_idx, :])
                nats.append(nat)

            # 2. xbar transpose -> attf [128p, FT, XCHUNK]; slice c holds f rows 128c..128c+127
            attf = attf_p.tile([128, FT, XCHUNK], BF16, tag="attf", name=f"attf_{c}")
            for i in range(S_PER_CHUNK):
                nc.sync.dma_start(out=attf[:, :, i * 128:(i + 1) * 128],
                                  in_=nats[i][:], transpose=True)

            # 3. MM1 + relu -> attT tiles [128p(r), XCHUNK] bf16
            attT_tiles = []
            for rb in range(RT):
                ps1 = ps_mm1.tile([128, XCHUNK], F32, tag="mm1", name=f"ps1_{c}_{rb}")
                for f in range(FT):
                    nc.tensor.matmul(ps1[:], w_ae[:, f, rb * 128:(rb + 1) * 128],
                                     attf[:, f, :], start=(f == 0), stop=(f == FT - 1))
                at_sb = attT_p.tile([128, XCHUNK], BF16, tag="attT", name=f"attT_{c}_{rb}")
                nc.scalar.activation(at_sb[:], ps1[:], AF.Relu,
                                     bias=b_ae[:, rb:rb + 1], scale=1.0)
                attT_tiles.append(at_sb)

            # 4. MM2 -> +h_eT bcast -> tanh(+b_c) -> hAT bf16
            hat_tiles = []
            for ab in range(AT):
                ps2 = ps_mm2.tile([128, XCHUNK], F32, tag="mm2", name=f"ps2_{c}_{ab}")
                for rb in range(RT):
                    nc.tensor.matmul(ps2[:], w_c[:, rb, ab * 128:(ab + 1) * 128],
                                     attT_tiles[rb][:], start=(rb == 0), stop=(rb == RT - 1))
                tmp = small_p.tile([128, XCHUNK], BF16, tag="hatmp", name=f"hatmp_{c}_{ab}")
                nc.vector.tensor_tensor(
                    out=tmp[:].rearrange("p (s b) -> p s b", s=S_PER_CHUNK),
                    in0=ps2[:].rearrange("p (s b) -> p s b", s=S_PER_CHUNK),
                    in1=h_eT[:, ab, :].unsqueeze(1).broadcast_to([128, S_PER_CHUNK, B_LOC]),
                    op=ALU.add)
                ht = hat_p.tile([128, XCHUNK], BF16, tag="hat", name=f"hat_{c}_{ab}")
                nc.scalar.activation(ht[:], tmp[:], AF.Tanh,
                                     bias=b_c[:, ab:ab + 1], scale=1.0)
                hat_tiles.append(ht)

            # 5. logits row -> col transposes -> logits_sb[:, 1+4c : 1+4c+4]
            ps_l = ps_small.tile([1, XCHUNK], F32, tag="pssm", name=f"ps_l_{c}")
            for ab in range(AT):
                nc.tensor.matmul(ps_l[:], wal[:, ab:ab + 1], hat_tiles[ab][:],
                                 start=(ab == 0), stop=(ab == AT - 1))
            lr = small_p.tile([1, XCHUNK], F32, tag="lrow", name=f"lr_{c}")
            nc.vector.tensor_copy(lr[:], ps_l[:])
            ps_cc = ps_small.tile([128, S_PER_CHUNK], F32, tag="pssm", name=f"ps_cc_{c}")
            for i in range(S_PER_CHUNK):
                nc.tensor.transpose(ps_cc[:, i:i + 1], lr[:, i * 128:(i + 1) * 128],
                                    ident_f32[0:1, 0:1])
            nc.vector.tensor_copy(
                logits_sb[:, 1 + c * S_PER_CHUNK: 1 + (c + 1) * S_PER_CHUNK], ps_cc[:])

            # 6. xbar attT -> att[x, r] per-chunk tile (for cHat), batched 4 s-tiles/op
            axc = attx_p.tile([128, S_PER_CHUNK, R], BF16, tag="attx", name=f"attx_{c}")
            for rb in range(RT):
                nc.sync.dma_start(out=axc[:, :, rb * 128:(rb + 1) * 128],
                                    in_=attT_tiles[rb][:], transpose=True)
            att_x_chunks.append(axc)

        # ---- softmax over 97 slots (f32) ----
        mx = soft_p.tile([B_LOC, 1], F32, tag="soft", name="mx")
        nc.vector.tensor_reduce(out=mx[:], in_=logits_sb[:], op=ALU.max,
                                axis=mybir.AxisListType.X)
        shifted = soft_p.tile([B_LOC, 1 + S], F32, tag="soft", name="shifted")
        nc.vector.tensor_scalar(out=shifted[:], in0=logits_sb[:], scalar1=mx[:],
                                scalar2=None, op0=ALU.subtract)
        expd = soft_p.tile([B_LOC, 1 + S], F32, tag="soft", name="expd")
        nc.scalar.activation(expd[:], shifted[:], AF.Exp)
        ssum = soft_p.tile([B_LOC, 1], F32, tag="soft", name="ssum")
        nc.vector.tensor_reduce(out=ssum[:], in_=expd[:], op=ALU.add,
                                axis=mybir.AxisListType.X)
        rin = soft_p.tile([B_LOC, 1], F32, tag="soft", name="rin")
        nc.vector.reciprocal(rin[:], ssum[:])
        alpha = cp.tile([B_LOC, 1 + S], F32, tag="alpha", name="alpha")
        nc.vector.tensor_scalar(out=alpha[:], in0=expd[:], scalar1=rin[:],
                                scalar2=None, op0=ALU.mult)

        # ---- cHat: 97 accumulating diag matmuls ----
        ps_cH = ps_chat.tile([B_LOC, R], F32, name="ps_cH")
        ms0 = msel_p.tile([128, 128], BF16, tag="msel", name="ms0")
        nc.vector.tensor_scalar(out=ms0[:], in0=ident[:], scalar1=alpha[:, 0:1],
                                scalar2=None, op0=ALU.mult)
        nc.tensor.matmul(ps_cH[:], ms0[:], sent_bf[:], start=True, stop=False)
        for t in range(S):
            ms = msel_p.tile([128, 128], BF16, tag="msel", name=f"ms_{t}")
            nc.vector.tensor_scalar(out=ms[:], in0=ident[:], scalar1=alpha[:, t + 1:t + 2],
                                    scalar2=None, op0=ALU.mult)
            nc.tensor.matmul(ps_cH[:], ms[:], att_x_chunks[t // S_PER_CHUNK][:, t % S_PER_CHUNK, :],
                             start=False, stop=(t == S - 1))

        # ---- final: out = tanh((cHat + h) @ W_oT + b_o) ----
        atten_bf = cp.tile([B_LOC, R], BF16, tag="atten", name="atten_bf")
        nc.vector.tensor_tensor(out=atten_bf[:], in0=ps_cH[:], in1=h_f32[:], op=ALU.add)
        attenT = cp.tile([128, RT, B_LOC], BF16, tag="attenT", name="attenT")
        for rb in range(RT):
            ptf = ps_small.tile([128, 128], BF16, tag="pssm", name=f"ptf_{rb}")
            nc.tensor.transpose(ptf[:], atten_bf[:, rb * 128:(rb + 1) * 128], ident[:])
            nc.vector.tensor_copy(attenT[:, rb, :], ptf[:])
        for ob in range(RT):
            pso = ps_small.tile([128, B_LOC], F32, tag="pssm", name=f"pso_{ob}")
            for rb in range(RT):
                nc.tensor.matmul(pso[:], w_o[:, rb, ob * 128:(ob + 1) * 128], attenT[:, rb, :],
                                 start=(rb == 0), stop=(rb == RT - 1))
            otmp = small_p.tile([128, B_LOC], F32, tag="otmp", name=f"otmp_{ob}")
            nc.scalar.activation(otmp[:], pso[:], AF.Tanh,
                                 bias=b_o[:, ob:ob + 1], scale=1.0)
            ptb = ps_small.tile([128, 128], F32, tag="pssm", name=f"ptb_{ob}")
            nc.tensor.transpose(ptb[:], otmp[:], ident_f32[:])
            ostg = small_p.tile([128, 128], F32, tag="ostg", name=f"ostg_{ob}")
            nc.vector.tensor_copy(ostg[:], ptb[:])
            nc.gpsimd.dma_start(out=out_d[:, ob * 128:(ob + 1) * 128], in_=ostg[:])

    nc.compile()
    return nc


# ---------------- host side ----------------
_NC_CACHE = None


def _get_nc():
    global _NC_CACHE
    if _NC_CACHE is None:
        _NC_CACHE = build_nc()
    return _NC_CACHE


def prep_shared(W_ae, b_ae, W_c, b_c, W_s, b_s, W_h, b_h, W_al, b_al, W_o, b_o):
    bf = ml_dtypes.bfloat16

    def wt(w, nt):  # [p, t, n] = w.T[128*t + p, n]
        wT = np.ascontiguousarray(np.asarray(w, np.float32).T)
        return np.ascontiguousarray(
            wT.reshape(nt, 128, wT.shape[1]).transpose(1, 0, 2)).astype(bf)

    def bt(b, nt):  # [p, t] = b[128*t + p]
        return np.ascontiguousarray(
            np.asarray(b, np.float32).reshape(nt, 128).T).astype(np.float32)

    return {
        "w_ae_t": wt(W_ae, FT),
        "w_c_t": wt(W_c, RT),
        "w_s_t": wt(W_s, RT),
        "w_h_t": wt(W_h, RT),
        "w_o_t": wt(W_o, RT),
        "w_al": np.ascontiguousarray(
            np.asarray(W_al, np.float32)[0].reshape(AT, 128).T).astype(bf),
        "b_ae": bt(b_ae, RT),
        "b_c": bt(b_c, AT),
        "b_s": bt(b_s, AT),
        "b_h": bt(b_h, AT),
        "b_o": bt(b_o, RT),
        "ident": np.eye(128, dtype=bf),
        "ident_f32": np.eye(128, dtype=np.float32),
    }


def make_in_maps(h, sentinel, att_feats, shared):
    h = np.asarray(h, np.float32)
    sentinel = np.asarray(sentinel, np.float32)
    att_feats = np.asarray(att_feats, np.float32)
    in_maps = []
    for i in range(NCORES):
        sl = slice(i * B_LOC, (i + 1) * B_LOC)
        m = dict(shared)
        m["h"] = np.ascontiguousarray(h[sl])
        m["sentinel"] = np.ascontiguousarray(sentinel[sl])
        m["att_feats"] = np.ascontiguousarray(att_feats[sl])
        in_maps.append(m)
    return in_maps


def kernel(h, sentinel, att_feats, W_ae, b_ae, W_c, b_c, W_s, b_s,
           W_h, b_h, W_al, b_al, W_o, b_o):
    shared = prep_shared(W_ae, b_ae, W_c, b_c, W_s, b_s, W_h, b_h, W_al, b_al, W_o, b_o)
    in_maps = make_in_maps(h, sentinel, att_feats, shared)
    nc = _get_nc()
    from concourse.bass_utils import run_bass_kernel_spmd
    res = run_bass_kernel_spmd(nc, in_maps, core_ids=list(range(NCORES)))
    out = np.concatenate([res.results[i]["out"] for i in range(NCORES)], axis=0)
    return np.ascontiguousarray(out.astype(np.float32))


if __name__ == "__main__":
    build_nc()
    print("built ok")
